# revision 1
# baseline (speedup 1.0000x reference)
"""Trainium2 Bass kernel for nn_MfdFC (weighted Frechet mean on S^7).

Self-contained: kernel(x, w) -> [8,1024,32,8] float32.

Math (per point b, out-channel o, 3 fixed iterations):
  c[o,i] = <a_o, x_i>  (clipped)
  theta  = arccos(c) = 2*arctan(z) + pi/2,  z = (w - s)/(w + s),
           w = 1 - c, s = sqrt(1 - c^2)            (z in [-1,1] -> HW arctan ok)
  factor = theta / s ; fw = wn[i,o] * factor
  grad1[o,:] = sum_i fw * x_i ; gs = <a_o, grad1_o>  (== sum_i fw*c)
  grad = grad1 - gs*a_o ; n2 = |grad|^2
  a_o <- cos(n)*a_o + sinc(n)*grad   (cos/sinc via Taylor in n2; n <= pi)

Layout scheme per core: 1024 points = 4 megagroups (mg) x 4 supergroups (sg)
x 64 points; point = (g,j,p) each in [0,4).  Layouts:
  XC  [(j,i), (sg,g,p,d)]   grad-mm moving operand
  X4  [(g,p,d), (sg,j,i)]   inner-mm stationary
  ABD [(g,p,d), (sg,j,p',o)] block-diag moving operand (p'==p blocks)
  IN/FW [(j,i), (g,sg,p,o)] inner results / weighted factors
  GR/AE [(g,o), (sg,j,p,d)] grad results / current a
PE runs 16-way 32x32 tile-packed matmuls (tile_position).
"""

import numpy as np

B, L, CIN, COUT, D = 8, 1024, 32, 32, 8
NCORES = 8
PTS_PER_CORE = (B * L) // NCORES  # 1024
N_MG = 4          # megagroups per core
N_SG = 4          # supergroups per megagroup
N_ITER = 3
CLIP = np.float32(1.0 - 1e-7)
WMIN = np.float32(1e-7)
WMAX = np.float32(2.0 - 1e-7)
PI = float(np.pi)

_CACHE = {}


# --------------------------------------------------------------------------
# host-side layout packing
# --------------------------------------------------------------------------
def _host_prep(x, w):
    xs = np.ascontiguousarray(x, dtype=np.float32).reshape(B * L, CIN, D)
    wf = np.asarray(w, dtype=np.float32)
    wn = np.exp(wf)
    wn = wn / wn.sum(axis=0, keepdims=True)          # [CIN(i), COUT(o)]
    wn2 = (2.0 * wn).astype(np.float32)

    # WN2F [(j,i), (g,sg,p,o)] = 2*wn[i,o]
    wn2f = np.zeros((128, 2048), np.float32)
    i_idx = np.arange(128) % 32
    wn2f[:, :] = wn2[i_idx][:, None, None, None, :].reshape(128, -1).repeat(1, axis=0) \
        if False else np.tile(wn2[i_idx][:, None, :], (1, 64, 1)).reshape(128, 2048)

    # MASKC [(g,p,d), (sg,j,p',o)] = 1 if p' == p
    maskc = np.zeros((4, 4, 8, N_SG, 4, 4, 32), np.float32)
    for p in range(4):
        maskc[:, p, :, :, :, p, :] = 1.0
    maskc = maskc.reshape(128, 2048)

    per_core = []
    for c in range(NCORES):
        pts = xs[c * PTS_PER_CORE:(c + 1) * PTS_PER_CORE]  # [1024, 32, 8]
        # index: pt = ((mg*4+sg)*64) + g*16 + j*4 + p
        v = pts.reshape(N_MG, N_SG, 4, 4, 4, CIN, D)  # [mg,sg,g,j,p,i,d]

        # XC [mg, 32j+i, sg*128 + g*32 + p*8 + d]
        xc = np.ascontiguousarray(
            v.transpose(0, 3, 5, 1, 2, 4, 6)  # mg, j, i, sg, g, p, d
        ).reshape(N_MG, 128, 512)

        # X4 [mg, 32g+8p+d, sg*128 + j*32 + i]
        x4 = np.ascontiguousarray(
            v.transpose(0, 2, 4, 6, 1, 3, 5)  # mg, g, p, d, sg, j, i
        ).reshape(N_MG, 128, 512)

        # ABD0 [mg, 32g+8p+d, sg*512 + j*128 + p'*32 + o] = x[pt(g,j,p'),0,d]
        abd0 = np.zeros((N_MG, 4, 4, 8, N_SG, 4, 4, 32), np.float32)
        x0 = v[:, :, :, :, :, 0, :]  # [mg,sg,g,j,p,d]
        for p in range(4):
            # dest p-partition band (axis 2 idx of (g,P,d)) gets block at p'=p
            abd0[:, :, p, :, :, :, p, :] = (
                x0[:, :, :, :, p, :].transpose(0, 2, 4, 1, 3)[:, :, :, :, :, None]
            )
        abd0 = abd0.reshape(N_MG, 128, 2048)

        # X0E [mg, 32g+o, sg*128 + j*32 + p*8 + d] = x[pt,0,d] (all o)
        x0e = np.ascontiguousarray(
            np.broadcast_to(
                x0.transpose(0, 2, 1, 3, 4, 5)[:, :, None, :, :, :, :],
                (N_MG, 4, 32, N_SG, 4, 4, D),
            )
        ).reshape(N_MG, 128, 512)

        per_core.append({"xc": xc, "x4": x4, "abd0": abd0, "x0e": x0e,
                         "wn2": wn2f, "maskc": maskc})
    return per_core


def _host_unpack(outs):
    """outs: list of 8 arrays [N_MG, 128, 512] in AE layout -> [B,L,COUT,D]."""
    res = np.empty((B * L, COUT, D), np.float32)
    for c, oe in enumerate(outs):
        v = oe.reshape(N_MG, 4, 32, N_SG, 4, 4, D)  # mg, g, o, sg, j, p, d
        v = v.transpose(0, 3, 1, 4, 5, 2, 6)        # mg, sg, g, j, p, o, d
        res[c * PTS_PER_CORE:(c + 1) * PTS_PER_CORE] = v.reshape(
            PTS_PER_CORE, COUT, D)
    return res.reshape(B, L, COUT, D)


# --------------------------------------------------------------------------
# bass program
# --------------------------------------------------------------------------
def build_bass(n_mg=N_MG, n_sg=N_SG, n_iter=N_ITER, strip=True):
    import concourse.bass as bass
    import concourse.mybir as mybir
    from concourse.tile import TileContext
    from concourse.tile_rust import add_dep_helper

    f32 = mybir.dt.float32
    AF = mybir.ActivationFunctionType
    OP = mybir.AluOpType

    nc = bass.Bass()

    def act_raw(out, in_, func, scale=1.0):
        import concourse.mybir as _mb
        eng = nc.scalar
        bias_ap = nc.const_aps.scalar_like(0.0, in_)
        return eng.add_instruction(_mb.InstActivation(
            name=nc.get_next_instruction_name(), func=func,
            ins=[eng.lower_ap(in_), eng.lower_ap(bias_ap),
                 _mb.ImmediateValue(dtype=_mb.dt.float32, value=float(scale)),
                 _mb.ImmediateValue(dtype=_mb.dt.float32, value=0.0)],
            outs=[eng.lower_ap(out)]))

    def eng_fence(eng, deps):
        last = None
        for di in deps:
            if di is None:
                continue
            n = eng.nop(nofuse=True, hint="fence")
            add_dep_helper(n.ins, di.ins, reason="fence")
            last = n
        return last

    def pe_fence(deps):
        return eng_fence(nc.tensor, deps)

    xc_d = nc.dram_tensor("xc", [n_mg, 128, 512], f32, kind="ExternalInput")
    x4_d = nc.dram_tensor("x4", [n_mg, 128, 512], f32, kind="ExternalInput")
    abd0_d = nc.dram_tensor("abd0", [n_mg, 128, 2048], f32, kind="ExternalInput")
    x0e_d = nc.dram_tensor("x0e", [n_mg, 128, 512], f32, kind="ExternalInput")
    wn2_d = nc.dram_tensor("wn2", [128, 2048], f32, kind="ExternalInput")
    maskc_d = nc.dram_tensor("maskc", [128, 2048], f32, kind="ExternalInput")
    out_ds = [nc.dram_tensor(f"out{k}", [128, 512], f32, kind="ExternalOutput")
              for k in range(n_mg)]

    with TileContext(nc) as tc:
        with (
            tc.tile_pool(name="const", bufs=1) as constp,
            tc.tile_pool(name="io", bufs=4) as iop,
            tc.tile_pool(name="abdp", bufs=2) as abdp,
            tc.tile_pool(name="anewp", bufs=2) as anp,
            tc.tile_pool(name="chain", bufs=1) as chp,
            tc.tile_pool(name="small", bufs=1) as smp,
            tc.tile_pool(name="psum", bufs=1, space="PSUM") as psp,
        ):
            wn2 = constp.tile([128, 2048], f32)
            d_wn2 = nc.sync.dma_start(wn2[:], wn2_d[:])
            maskc = constp.tile([128, 2048], f32)
            d_maskc = nc.sync.dma_start(maskc[:], maskc_d[:])
            eng_fence(nc.sync, [d_maskc, d_wn2])

            _carry = {}
            _alldmas = []
            for mg in range(n_mg):
                xc4 = iop.tile([128, 512], f32, tag="xc4")
                x44 = iop.tile([128, 512], f32, tag="x44")
                abd0 = iop.tile([128, 2048], f32, tag="abd0")
                x0e = iop.tile([128, 512], f32, tag="x0e")
                d_xc = nc.sync.dma_start(xc4[:], xc_d[mg])
                d_x4 = nc.sync.dma_start(x44[:], x4_d[mg])
                d_abd0 = nc.sync.dma_start(abd0[:], abd0_d[mg])
                d_x0e = nc.sync.dma_start(x0e[:], x0e_d[mg])

                prev_abd = None
                ps = psp.tile([128, 4096], f32)  # banks: GR 0-3, IN 4-7

                aold = x0e
                lim = None
                lgmm = _carry.get("lgmm")
                at_prev = _carry.get("at")
                q_prev = _carry.get("q")
                w1_prev = _carry.get("w1")
                gr_readers_prev = _carry.get("grdr", [])
                for it in range(n_iter):
                    mov = abd0 if it == 0 else prev_abd

                    # ---- inner matmuls: 64 MMs (sg,g,j), N=128 ----
                    if it == 0:
                        fdeps = [d_x4, d_abd0, w1_prev] + gr_readers_prev
                    else:
                        fdeps = [scat_i, w1_prev]
                    fence = pe_fence(fdeps)
                    for sg in range(n_sg):
                        for g in range(4):
                            for j in range(4):
                                _mm = nc.tensor.matmul(
                                    ps[32 * j:32 * j + 32,
                                       2048 + g * 512 + sg * 128:
                                       2048 + g * 512 + sg * 128 + 128],
                                    x44[32 * g:32 * g + 32,
                                        sg * 128 + j * 32:sg * 128 + j * 32 + 32],
                                    mov[32 * g:32 * g + 32,
                                        sg * 512 + j * 128:sg * 512 + j * 128 + 128],
                                    tile_position=(32 * g, 32 * j),
                                )
                                if fence is not None:
                                    add_dep_helper(_mm.ins, fence.ins,
                                                   sync=False, reason="order")
                                lim = _mm

                    # ---- factor chain on [128, 2048] ----
                    inap = ps[:, 2048:4096]
                    eng_fence(nc.vector,
                              [lim, at_prev,
                               d_maskc if (mg == 0 and it == 0) else None])
                    w1 = chp.tile([128, 2048], f32, tag="t0")
                    w1_prev = nc.vector.tensor_scalar(w1[:], inap, -1.0, 1.0, OP.mult, OP.add)
                    wt = chp.tile([128, 2048], f32, tag="t1")
                    nc.vector.tensor_scalar(wt[:], w1[:], float(WMIN), float(WMAX),
                                            OP.max, OP.min)
                    eng_fence(nc.vector, [at_prev])
                    msq = chp.tile([128, 2048], f32, tag="t2")
                    msq_i = nc.vector.scalar_tensor_tensor(msq[:], wt[:], 2.0,
                                                           wt[:], OP.subtract,
                                                           OP.mult)
                    rs = chp.tile([128, 2048], f32, tag="t3")
                    eng_fence(nc.scalar, [q_prev])
                    rs_i = act_raw(rs[:], msq[:], AF.Rsqrt, scale=-1.0)
                    eng_fence(nc.vector, [msq_i, rs_i])
                    s = chp.tile([128, 2048], f32, tag="t0")
                    s_i = nc.vector.scalar_tensor_tensor(s[:], msq[:], -1.0,
                                                         rs[:], OP.mult, OP.mult)
                    eng_fence(nc.vector, [s_i, rs_i, lgmm])
                    num = chp.tile([128, 2048], f32, tag="t2")
                    nc.vector.tensor_tensor(num[:], wt[:], s[:], OP.subtract)
                    den = chp.tile([128, 2048], f32, tag="t4")
                    den_i = nc.vector.tensor_tensor(den[:], wt[:], s[:], OP.add)
                    eng_fence(nc.scalar, [den_i, s_i])
                    rdq = chp.tile([128, 2048], f32, tag="t0")
                    rdq_i = act_raw(rdq[:], den[:], AF.Rsqrt)
                    eng_fence(nc.vector, [rdq_i, den_i])
                    rd = chp.tile([128, 2048], f32, tag="t1")
                    nc.vector.tensor_tensor(rd[:], rdq[:], rdq[:], OP.mult)
                    z = chp.tile([128, 2048], f32, tag="t0")
                    nc.vector.tensor_tensor(z[:], num[:], rd[:], OP.mult)
                    at = chp.tile([128, 2048], f32, tag="t2")
                    at_prev = nc.scalar.activation(at[:], z[:], AF.Arctan)
                    q = chp.tile([128, 2048], f32, tag="t1")
                    q_prev = nc.vector.scalar_tensor_tensor(q[:], at[:], PI / 4,
                                                            rs[:], OP.add,
                                                            OP.mult)
                    fw = chp.tile([128, 2048], f32, tag="t4")
                    fw_i = nc.vector.tensor_tensor(fw[:], q[:], wn2[:], OP.mult)

                    # ---- grad matmuls: 256 MMs, N=8 ----
                    gdeps = [fw_i] + gr_readers_prev
                    if it == 0:
                        gdeps.append(d_xc)
                    fence2 = pe_fence(gdeps)
                    for sg in range(n_sg):
                        for j in range(4):
                            for g in range(4):
                                for p in range(4):
                                    _gm = nc.tensor.matmul(
                                        ps[32 * g:32 * g + 32,
                                           j * 512 + sg * 32 + p * 8:
                                           j * 512 + sg * 32 + p * 8 + 8],
                                        fw[32 * j:32 * j + 32,
                                           g * 512 + sg * 128 + p * 32:
                                           g * 512 + sg * 128 + p * 32 + 32],
                                        xc4[32 * j:32 * j + 32,
                                            sg * 128 + g * 32 + p * 8:
                                            sg * 128 + g * 32 + p * 8 + 8],
                                        tile_position=(32 * j, 32 * g),
                                    )
                                    if fence2 is not None:
                                        add_dep_helper(_gm.ins, fence2.ins,
                                                       sync=False, reason="order")
                                    lgmm = _gm

                    # ---- exp-map chain ----
                    # GR free order (j, sg, p, d) -> view as (sg, j, p, d)
                    gr = ps[:, 0:2048].rearrange("p (j w s q) -> p w s j q",
                                                 j=4, w=4, s=4, q=32)[:, 0]
                    ao = aold[:].rearrange("p (s j q) -> p s j q", s=4, j=4, q=32)
                    eng_fence(nc.vector,
                              [lgmm, d_x0e if it == 0 else None])
                    ag = smp.tile([128, 512], f32, tag="ag")
                    agv = ag[:].rearrange("p (s j q) -> p s j q", s=4, j=4, q=32)
                    ag_i = nc.vector.tensor_tensor(agv, ao, gr, OP.mult)
                    gs = smp.tile([128, 64], f32, tag="gs")
                    gs_i = nc.vector.tensor_reduce(
                        gs[:], ag[:].rearrange("p (k d) -> p k d", d=8),
                        mybir.AxisListType.X, OP.add)
                    gsb = gs[:].rearrange("p (k o) -> p k o", o=1).broadcast_to((128, 64, 8))
                    aok = aold[:].rearrange("p (k d) -> p k d", d=8)
                    eng_fence(nc.vector, [gs_i, d_x0e if it == 0 else None])
                    t2 = smp.tile([128, 512], f32, tag="t2")
                    t2v = t2[:].rearrange("p (k d) -> p k d", d=8)
                    nc.vector.scalar_tensor_tensor(t2v, gsb, -1.0, aok,
                                                   OP.mult, OP.mult)
                    grad = smp.tile([128, 512], f32, tag="grad")
                    gradv = grad[:].rearrange("p (s j q) -> p s j q", s=4, j=4, q=32)
                    grad_i = nc.vector.tensor_tensor(gradv, gr, t2[:].rearrange(
                        "p (s j q) -> p s j q", s=4, j=4, q=32), OP.add)
                    gr_readers_prev = [ag_i, grad_i]
                    eng_fence(nc.vector, [grad_i])
                    g2 = smp.tile([128, 512], f32, tag="g2")
                    nc.vector.tensor_tensor(g2[:], grad[:], grad[:], OP.mult)
                    n2 = smp.tile([128, 64], f32, tag="n2")
                    n2_i = nc.vector.tensor_reduce(
                        n2[:], g2[:].rearrange("p (k d) -> p k d", d=8),
                        mybir.AxisListType.X, OP.add)

                    # Taylor sinc (deg3) and cos (deg4) in u = n2
                    sc = smp.tile([128, 64], f32, tag="sc")
                    tmp = smp.tile([128, 64], f32, tag="tmp")
                    nc.vector.tensor_scalar(tmp[:], n2[:], -1.0 / 5040.0,
                                            1.0 / 120.0, OP.mult, OP.add)
                    nc.vector.scalar_tensor_tensor(sc[:], tmp[:], 0.0, n2[:],
                                                   OP.add, OP.mult)
                    nc.vector.tensor_scalar(tmp[:], sc[:], -1.0 / 6.0, None, OP.add)
                    nc.vector.scalar_tensor_tensor(sc[:], tmp[:], 0.0, n2[:],
                                                   OP.add, OP.mult)
                    nc.vector.tensor_scalar(sc[:], sc[:], 1.0, None, OP.add)

                    eng_fence(nc.vector, [n2_i])
                    cc = smp.tile([128, 64], f32, tag="cc")
                    tmp2 = smp.tile([128, 64], f32, tag="tmp2")
                    nc.vector.tensor_scalar(tmp2[:], n2[:], 1.0 / 40320.0,
                                            -1.0 / 720.0, OP.mult, OP.add)
                    nc.vector.scalar_tensor_tensor(cc[:], tmp2[:], 0.0, n2[:],
                                                   OP.add, OP.mult)
                    nc.vector.tensor_scalar(tmp2[:], cc[:], 1.0 / 24.0, None, OP.add)
                    nc.vector.scalar_tensor_tensor(cc[:], tmp2[:], 0.0, n2[:],
                                                   OP.add, OP.mult)
                    nc.vector.tensor_scalar(tmp2[:], cc[:], -1.0 / 2.0, None, OP.add)
                    nc.vector.scalar_tensor_tensor(cc[:], tmp2[:], 0.0, n2[:],
                                                   OP.add, OP.mult)
                    nc.vector.tensor_scalar(cc[:], cc[:], 1.0, None, OP.add)

                    scb = sc[:].rearrange("p (k o) -> p k o", o=1).broadcast_to((128, 64, 8))
                    ccb = cc[:].rearrange("p (k o) -> p k o", o=1).broadcast_to((128, 64, 8))
                    u1 = smp.tile([128, 512], f32, tag="u1")
                    nc.vector.scalar_tensor_tensor(
                        u1[:].rearrange("p (k d) -> p k d", d=8), scb, 0.0,
                        grad[:].rearrange("p (k d) -> p k d", d=8),
                        OP.add, OP.mult)
                    u2 = smp.tile([128, 512], f32, tag="u2")
                    u2_i = nc.vector.scalar_tensor_tensor(
                        u2[:].rearrange("p (k d) -> p k d", d=8), ccb, 0.0,
                        aok, OP.add, OP.mult)
                    eng_fence(nc.vector, [_carry.get("outdma")])
                    anew = anp.tile([128, 512], f32, tag="anew")
                    _lastanew = anew
                    anew_i = nc.vector.tensor_tensor(anew[:], u1[:], u2[:], OP.add)

                    if it < n_iter - 1:
                        tt = smp.tile([128, 512], f32, tag="tt")
                        nc.vector.transpose(tt[:], anew[:])
                        # ABD block-diag = bcast(tt) * mask (one DVE op)
                        abd_c = abdp.tile([128, 2048], f32, tag="abd")
                        ttb = tt[:].rearrange("e (s j o) -> e s j o",
                                              s=4, j=4).rearrange(
                            "e s j (b o) -> e s j b o", b=1).broadcast_to(
                            (128, 4, 4, 4, 32))
                        scat_i = nc.vector.tensor_tensor(
                            abd_c[:].rearrange("e (s j b o) -> e s j b o",
                                               s=4, j=4, b=4),
                            ttb,
                            maskc[:].rearrange("e (s j b o) -> e s j b o",
                                               s=4, j=4, b=4),
                            OP.mult)
                        prev_abd = abd_c
                    else:
                        eng_fence(nc.sync,
                                  [anew_i, d_xc, d_x4, d_abd0, d_x0e])
                        _carry["outdma"] = nc.sync.dma_start(out_ds[mg][:], anew[:])
                        _alldmas.append(_carry["outdma"])
                    aold = anew
                _carry["w1"] = w1_prev
                _carry["grdr"] = gr_readers_prev
                _carry["lgmm"] = lgmm
                _carry["at"] = at_prev
                _carry["q"] = q_prev
            eng_fence(nc.sync,
                      _alldmas + [lgmm, at_prev, u2_i, anew_i, w1_prev])

    if strip:
        _strip_redundant_pe_waits(nc)
    return nc


def _strip_redundant_pe_waits(nc):
    """HW wait-slot limits: Matmult=1, engine ops=2, DMA=2.  Drop (a) waits
    on the instruction's own engine semaphore (in-order completion makes them
    redundant), (b) waits already covered by an earlier same-engine-queue
    wait (the fence NoOps inserted before each matmul phase)."""
    eng_sem = {"DVE": "DVE_", "Activation": "Activation_", "Pool": "Pool_",
               "PE": "PE_", "SP": "SP_"}
    covered = {}   # (engine, sem id) -> max waited value
    for inst in nc.all_instructions():
        eng = getattr(inst, "engine", None)
        ename = str(eng).split(".")[-1] if eng is not None else ""
        si = inst.sync_info
        if si is None:
            continue
        waits = list(si.on_wait or [])
        if not waits:
            continue
        tname = type(inst).__name__
        if tname in ("InstDrain", "InstEventSemaphore", "InstNoOp"):
            limit = 99
        else:
            limit = 1
        changed = False
        # (a) self-engine waits
        pref = eng_sem.get(ename) if ename in ("DVE", "Activation") else None
        if pref and tname != "InstDMACopy":
            nw = [x for x in waits if not x.ant_name.startswith(pref)]
            if len(nw) != len(waits):
                waits, changed = nw, True
        # (b) coverage strip when over limit
        if len(waits) > limit and ename in eng_sem:
            nw = [x for x in waits
                  if covered.get((ename, x.id), -(10 ** 9)) < x.wait_value]
            if len(nw) != len(waits):
                waits, changed = nw, True
            if len(nw) > limit:
                raise RuntimeError(
                    f"{tname} {inst.name} ({ename}) has {len(nw)} uncovered "
                    f"waits: {[x.ant_name for x in nw]}")
        if changed:
            si.on_wait = waits
        for x in waits:
            if ename in eng_sem and covered.get((ename, x.id), -(10 ** 9)) < x.wait_value:
                covered[(ename, x.id)] = x.wait_value


# --------------------------------------------------------------------------
# entry point
# --------------------------------------------------------------------------
def kernel(x, w):
    from concourse.bass_utils import run_bass_kernel_spmd

    per_core = _host_prep(x, w)
    if "nc" not in _CACHE:
        _CACHE["nc"] = build_bass()
    nc = _CACHE["nc"]
    in_maps = [per_core[c] for c in range(NCORES)]
    res = run_bass_kernel_spmd(nc, in_maps, list(range(NCORES)))
    _CACHE["exec_time_ns"] = getattr(res, "exec_time_ns", None)
    outs = [np.stack([res.results[c][f"out{k}"] for k in range(N_MG)])
            for c in range(NCORES)]
    return _host_unpack(outs)


if __name__ == "__main__":
    import sys
    sys.path.insert(0, "/root/problem")
    import reference

    inputs = reference.setup_inputs()
    out = kernel(**{k: np.asarray(v) for k, v in inputs.items()})
    print("kernel output shape:", out.shape, out.dtype)



# revision 41
# speedup vs baseline: 1.8088x; 1.8088x over previous
"""Trainium2 Bass kernel for nn_MfdFC (weighted Frechet mean on S^7).

Self-contained: kernel(x, w) -> [8,1024,32,8] float32.

Math (per point b, out-channel o, 3 fixed iterations):
  c[o,i] = <a_o, x_i>  (clipped to [-CLIP, CLIP])
  rs = 1/sqrt(1-c^2) = 1/s ; tn = (c-1)*rs = -tan(theta/2)
  at = arctan(tn) = -theta/2  (HW arctan accurate over full range)
  q  = 2*at*rs = -theta/s ; fw = q * (-wn) = +wn*theta/s
  gr[o,:] = sum_i fw * x_i = grad1 ; gs = <a_o, gr>
  n2 = |gr|^2 - gs^2 ; anew = (cos(n) - sinc(n)*gs)*a + sinc(n)*gr
  (cos/sinc via Taylor in n2)

Iteration 0 exploits a == x0 for ALL o: factor f[pt,i] is o-independent, so
c0 is computed elementwise+reduce (no PE), and grad1 = wn^T @ (f*x) becomes
16 big matmuls with the wn matrix stationary.

Layout per core: 1024 points = 4 megagroups (mg) x (sg,g,j,p) each in [0,4).
  XC  [(j,i), (sg,g,p,d)]   grad-mm moving operand / it0 elementwise
  X4  [(g,p,d), (sg,j,i)]   inner-mm stationary
  X0C [(j,i), (sg,g,p,d)]   x0 broadcast over i (it0)
  ABD [(g,p,d), (sg,j,p',o)] block-diag moving operand (p'==p blocks)
  IN/FW [(j,i), (g,sg,p,o)] inner results / weighted factors
  GR/AE [(g,o), (sg,j,p,d)] grad results / current a
PE runs 16-way 32x32 tile-packed matmuls (tile_position).
"""

import numpy as np

B, L, CIN, COUT, D = 8, 1024, 32, 32, 8
NCORES = 8
PTS_PER_CORE = (B * L) // NCORES  # 1024
N_MG = 4          # megagroups per core
N_SG = 4          # supergroups per megagroup
N_ITER = 3
CLIP = np.float32(1.0 - 1e-7)
PI = float(np.pi)

_CACHE = {}


# --------------------------------------------------------------------------
# host-side layout packing
# --------------------------------------------------------------------------
def _host_prep(x, w):
    xs = np.ascontiguousarray(x, dtype=np.float32).reshape(B * L, CIN, D)
    wf = np.asarray(w, dtype=np.float32)
    wn = np.exp(wf)
    wn = wn / wn.sum(axis=0, keepdims=True)          # [CIN(i), COUT(o)]
    wnneg = (-wn).astype(np.float32)

    i_idx = np.arange(128) % 32
    # WNF [(j,i), (g,sg,p,o)] = -wn[i,o]
    wnf = np.tile(wnneg[i_idx][:, None, :], (1, 64, 1)).reshape(128, 2048)
    # WN4 [(j,i), o] = -wn[i,o]
    wn4 = np.ascontiguousarray(wnneg[i_idx])  # [128, 32]

    # MASKC [(g,p,d), (sg,j,p',o)] = 1 if p' == p
    maskc = np.zeros((4, 4, 8, N_SG, 4, 4, 32), np.float32)
    for p in range(4):
        maskc[:, p, :, :, :, p, :] = 1.0
    maskc = maskc.reshape(128, 2048)

    per_core = []
    for c in range(NCORES):
        pts = xs[c * PTS_PER_CORE:(c + 1) * PTS_PER_CORE]  # [1024, 32, 8]
        # index: pt = ((mg*4+sg)*64) + g*16 + j*4 + p
        v = pts.reshape(N_MG, N_SG, 4, 4, 4, CIN, D)  # [mg,sg,g,j,p,i,d]

        # XC [mg, 32j+i, sg*128 + g*32 + p*8 + d]
        xc = np.ascontiguousarray(
            v.transpose(0, 3, 5, 1, 2, 4, 6)  # mg, j, i, sg, g, p, d
        ).reshape(N_MG, 128, 512)

        # X4 [mg, 32g+8p+d, sg*128 + j*32 + i]
        x4 = np.ascontiguousarray(
            v.transpose(0, 2, 4, 6, 1, 3, 5)  # mg, g, p, d, sg, j, i
        ).reshape(N_MG, 128, 512)

        x0 = v[:, :, :, :, :, 0, :]  # [mg,sg,g,j,p,d]
        # X0C [mg, 32j+i, sg*128+g*32+p*8+d] = x0 (bcast over i)
        x0t = np.ascontiguousarray(
            x0.transpose(0, 3, 1, 2, 4, 5)).reshape(N_MG, 4, 512)
        x0c = np.ascontiguousarray(
            np.broadcast_to(x0t[:, :, None, :], (N_MG, 4, 32, 512))
        ).reshape(N_MG, 128, 512)

        # X0E [mg, 32g+o, sg*128 + j*32 + p*8 + d] = x[pt,0,d] (all o)
        x0e = np.ascontiguousarray(
            np.broadcast_to(
                x0.transpose(0, 2, 1, 3, 4, 5)[:, :, None, :, :, :, :],
                (N_MG, 4, 32, N_SG, 4, 4, D),
            )
        ).reshape(N_MG, 128, 512)

        per_core.append({"xc": xc, "x4": x4, "x0c": x0c, "x0e": x0e,
                         "wnf": wnf, "wn4": wn4, "maskc": maskc})
    return per_core


def _host_unpack(outs):
    """outs: list of 8 arrays [N_MG, 128, 512] in AE layout -> [B,L,COUT,D]."""
    res = np.empty((B * L, COUT, D), np.float32)
    for c, oe in enumerate(outs):
        v = oe.reshape(N_MG, 4, 32, N_SG, 4, 4, D)  # mg, g, o, sg, j, p, d
        v = v.transpose(0, 3, 1, 4, 5, 2, 6)        # mg, sg, g, j, p, o, d
        res[c * PTS_PER_CORE:(c + 1) * PTS_PER_CORE] = v.reshape(
            PTS_PER_CORE, COUT, D)
    return res.reshape(B, L, COUT, D)


# --------------------------------------------------------------------------
# bass program
# --------------------------------------------------------------------------
def build_bass(n_mg=N_MG, n_sg=N_SG, strip=True, debug=False, split=True):
    import concourse.bass as bass
    import concourse.mybir as mybir
    from concourse.tile import TileContext
    from concourse.tile_rust import add_dep_helper

    f32 = mybir.dt.float32
    AF = mybir.ActivationFunctionType
    OP = mybir.AluOpType

    nc = bass.Bass()

    def act_raw(out, in_, func, scale=1.0, bias=0.0):
        eng = nc.scalar
        if func in (AF.Copy, AF.Reciprocal):
            bias_arg = mybir.ImmediateValue(dtype=mybir.dt.float32,
                                            value=float(bias))
        else:
            bias_arg = eng.lower_ap(nc.const_aps.scalar_like(float(bias), in_))
        return eng.add_instruction(mybir.InstActivation(
            name=nc.get_next_instruction_name(), func=func,
            ins=[eng.lower_ap(in_), bias_arg,
                 mybir.ImmediateValue(dtype=mybir.dt.float32, value=float(scale)),
                 mybir.ImmediateValue(dtype=mybir.dt.float32, value=0.0)],
            outs=[eng.lower_ap(out)]))

    def eng_fence(eng, deps):
        last = None
        for di in deps:
            if di is None:
                continue
            n = eng.nop(nofuse=True, hint="fence")
            add_dep_helper(n.ins, di.ins, reason="fence")
            last = n
        return last

    def pe_fence(deps):
        return eng_fence(nc.tensor, deps)

    xc_d = nc.dram_tensor("xc", [n_mg, 128, 512], f32, kind="ExternalInput")
    x4_d = nc.dram_tensor("x4", [n_mg, 128, 512], f32, kind="ExternalInput")
    x0c_d = nc.dram_tensor("x0c", [n_mg, 128, 512], f32, kind="ExternalInput")
    x0e_d = nc.dram_tensor("x0e", [n_mg, 128, 512], f32, kind="ExternalInput")
    wnf_d = nc.dram_tensor("wnf", [128, 2048], f32, kind="ExternalInput")
    wn4_d = nc.dram_tensor("wn4", [128, 32], f32, kind="ExternalInput")
    maskc_d = nc.dram_tensor("maskc", [128, 2048], f32, kind="ExternalInput")
    out_ds = [nc.dram_tensor(f"out{k}", [128, 512], f32, kind="ExternalOutput")
              for k in range(n_mg)]

    with TileContext(nc) as tc:
        with (
            tc.tile_pool(name="const", bufs=1) as constp,
            tc.tile_pool(name="io", bufs=3) as iop,
            tc.tile_pool(name="abdp", bufs=2) as abdp,
            tc.tile_pool(name="anewp", bufs=2) as anp,
            tc.tile_pool(name="chain", bufs=1) as chp,
            tc.tile_pool(name="small", bufs=1) as smp,
            tc.tile_pool(name="psum", bufs=1, space="PSUM") as psp,
        ):
            wnf = constp.tile([128, 2048], f32)
            d_wnf = nc.sync.dma_start(wnf[:], wnf_d[:])
            wn4 = constp.tile([128, 32], f32)
            d_wn4 = nc.sync.dma_start(wn4[:], wn4_d[:])
            maskc = constp.tile([128, 2048], f32)
            d_maskc = nc.sync.dma_start(maskc[:], maskc_d[:])
            eng_fence(nc.sync, [d_maskc, d_wnf, d_wn4])

            _carry = {}
            _alldmas = []
            _dbg = {}

            def dbg(name, ap, dep, shape):
                if not debug or name in _dbg:
                    return
                dt_ = nc.dram_tensor(f"dbg_{name}", list(shape), f32,
                                     kind="ExternalOutput")
                eng_fence(nc.sync, [dep])
                dd = nc.sync.dma_start(dt_[:], ap)
                _dbg[name] = dd
                _alldmas.append(dd)

            # ---------- exp-map chain (shared by all iterations) ----------
            def exp_chain(mg, it, aold, d_x0e, lgmm):
                """aold: AE tile; GR = psum bank 0 in AE order. anew out."""
                ps = _carry["ps"]
                gr = ps[:, 0:512]
                eng_fence(nc.scalar, [lgmm])
                grs = smp.tile([128, 512], f32, tag="grs")
                grs_i = act_raw(grs[:], gr, AF.Copy)
                grk = grs[:].rearrange("p (k d) -> p k d", d=8)
                eng_fence(nc.vector, [grs_i, d_x0e])
                ag = smp.tile([128, 512], f32, tag="ag")
                ag_i = nc.vector.tensor_tensor(ag[:], aold[:], grs[:], OP.mult)
                g2 = smp.tile([128, 512], f32, tag="g2")
                g2_i = act_raw(g2[:], grs[:], AF.Square)
                _carry["lastact"] = g2_i
                gs = smp.tile([128, 64], f32, tag="gs")
                gs_i = nc.vector.tensor_reduce(
                    gs[:], ag[:].rearrange("p (k d) -> p k d", d=8),
                    mybir.AxisListType.X, OP.add)
                sg2 = smp.tile([128, 64], f32, tag="sg2")
                eng_fence(nc.vector, [g2_i])
                nc.vector.tensor_reduce(
                    sg2[:], g2[:].rearrange("p (k d) -> p k d", d=8),
                    mybir.AxisListType.X, OP.add)
                gs2 = smp.tile([128, 64], f32, tag="gs2")
                nc.vector.tensor_tensor(gs2[:], gs[:], gs[:], OP.mult)
                n2 = smp.tile([128, 64], f32, tag="n2")
                n2_i = nc.vector.tensor_tensor(n2[:], sg2[:], gs2[:], OP.subtract)

                # Taylor sinc (deg3) and cos (deg4) in u = n2
                sc = smp.tile([128, 64], f32, tag="sc")
                tmp = smp.tile([128, 64], f32, tag="tmp")
                nc.vector.tensor_scalar(tmp[:], n2[:], -1.0 / 5040.0,
                                        1.0 / 120.0, OP.mult, OP.add)
                nc.vector.scalar_tensor_tensor(sc[:], tmp[:], 0.0, n2[:],
                                               OP.add, OP.mult)
                nc.vector.tensor_scalar(tmp[:], sc[:], -1.0 / 6.0, None, OP.add)
                nc.vector.scalar_tensor_tensor(sc[:], tmp[:], 0.0, n2[:],
                                               OP.add, OP.mult)
                nc.vector.tensor_scalar(sc[:], sc[:], 1.0, None, OP.add)

                eng_fence(nc.vector, [n2_i])
                cc = smp.tile([128, 64], f32, tag="cc")
                tmp2 = smp.tile([128, 64], f32, tag="tmp2")
                nc.vector.tensor_scalar(tmp2[:], n2[:], 1.0 / 40320.0,
                                        -1.0 / 720.0, OP.mult, OP.add)
                nc.vector.scalar_tensor_tensor(cc[:], tmp2[:], 0.0, n2[:],
                                               OP.add, OP.mult)
                nc.vector.tensor_scalar(tmp2[:], cc[:], 1.0 / 24.0, None, OP.add)
                nc.vector.scalar_tensor_tensor(cc[:], tmp2[:], 0.0, n2[:],
                                               OP.add, OP.mult)
                nc.vector.tensor_scalar(tmp2[:], cc[:], -1.0 / 2.0, None, OP.add)
                nc.vector.scalar_tensor_tensor(cc[:], tmp2[:], 0.0, n2[:],
                                               OP.add, OP.mult)
                nc.vector.tensor_scalar(cc[:], cc[:], 1.0, None, OP.add)

                # ca = cc - sc*gs
                scgs = smp.tile([128, 64], f32, tag="scgs")
                nc.vector.tensor_tensor(scgs[:], sc[:], gs[:], OP.mult)
                ca = smp.tile([128, 64], f32, tag="ca")
                nc.vector.tensor_tensor(ca[:], cc[:], scgs[:], OP.subtract)

                scb = sc[:].rearrange("p (k o) -> p k o",
                                      o=1).broadcast_to((128, 64, 8))
                cab = ca[:].rearrange("p (k o) -> p k o",
                                      o=1).broadcast_to((128, 64, 8))
                aok = aold[:].rearrange("p (k d) -> p k d", d=8)
                u1 = smp.tile([128, 512], f32, tag="u1")
                u1_i = nc.vector.scalar_tensor_tensor(
                    u1[:].rearrange("p (k d) -> p k d", d=8), scb, 0.0,
                    grk, OP.add, OP.mult)
                u2 = smp.tile([128, 512], f32, tag="u2")
                u2_i = nc.vector.scalar_tensor_tensor(
                    u2[:].rearrange("p (k d) -> p k d", d=8), cab, 0.0,
                    aok, OP.add, OP.mult)
                eng_fence(nc.vector, [_carry.get("outdma")])
                anew = anp.tile([128, 512], f32, tag="anew")
                anew_i = nc.vector.tensor_tensor(anew[:], u1[:], u2[:], OP.add)
                return anew, anew_i, [grs_i], u2_i

            def make_abd(anew, anew_i, first):
                tt = smp.tile([128, 512], f32, tag="tt")
                nc.vector.transpose(tt[:], anew[:])
                abd_c = abdp.tile([128, 2048], f32, tag="abd")
                ttb = tt[:].rearrange("e (s j o) -> e s j o",
                                      s=4, j=4).rearrange(
                    "e s j (b o) -> e s j b o", b=1).broadcast_to(
                    (128, 4, 4, 4, 32))
                if first:
                    eng_fence(nc.vector, [d_maskc])
                scat_i = nc.vector.tensor_tensor(
                    abd_c[:].rearrange("e (s j b o) -> e s j b o",
                                       s=4, j=4, b=4),
                    ttb,
                    maskc[:].rearrange("e (s j b o) -> e s j b o",
                                       s=4, j=4, b=4),
                    OP.mult)
                return abd_c, scat_i

            for mg in range(n_mg):
                xc4 = iop.tile([128, 512], f32, tag="xc4")
                x44 = iop.tile([128, 512], f32, tag="x44")
                x0c = iop.tile([128, 512], f32, tag="x0c")
                x0e = iop.tile([128, 512], f32, tag="x0e")
                d_xc = nc.sync.dma_start(xc4[:], xc_d[mg])
                d_x4 = nc.sync.dma_start(x44[:], x4_d[mg])
                d_x0c = nc.sync.dma_start(x0c[:], x0c_d[mg])
                d_x0e = nc.sync.dma_start(x0e[:], x0e_d[mg])

                ps = psp.tile([128, 4096], f32)  # GR bank 0; IN banks 4-7
                _carry["ps"] = ps

                # ================= iteration 0 =================
                eng_fence(nc.vector, [d_xc, d_x0c] + _carry.get("grdr", []))
                m0 = smp.tile([128, 512], f32, tag="m0")
                nc.vector.tensor_tensor(m0[:], xc4[:], x0c[:], OP.mult)
                c0 = smp.tile([128, 64], f32, tag="c0")
                nc.vector.tensor_reduce(
                    c0[:], m0[:].rearrange("p (k d) -> p k d", d=8),
                    mybir.AxisListType.X, OP.add)
                ccl0 = smp.tile([128, 64], f32, tag="ccl0")
                nc.vector.tensor_scalar(ccl0[:], c0[:], float(-CLIP),
                                        float(CLIP), OP.max, OP.min)
                c20 = smp.tile([128, 64], f32, tag="c20")
                c20_i = nc.vector.tensor_tensor(c20[:], ccl0[:], ccl0[:], OP.mult)
                rs0 = smp.tile([128, 64], f32, tag="rs0")
                eng_fence(nc.scalar, [c20_i, _carry.get("q")])
                rs0_i = act_raw(rs0[:], c20[:], AF.Rsqrt, scale=-1.0, bias=1.0)
                tn0 = smp.tile([128, 64], f32, tag="tn0")
                eng_fence(nc.vector, [rs0_i])
                tn0_i = nc.vector.scalar_tensor_tensor(tn0[:], ccl0[:], 1.0,
                                                       rs0[:], OP.subtract,
                                                       OP.mult)
                at0 = smp.tile([128, 64], f32, tag="at0")
                eng_fence(nc.scalar, [tn0_i])
                at0_i = act_raw(at0[:], tn0[:], AF.Arctan)
                q0 = smp.tile([128, 64], f32, tag="q0")
                eng_fence(nc.vector, [at0_i])
                nc.vector.scalar_tensor_tensor(q0[:], at0[:], 2.0, rs0[:],
                                               OP.mult, OP.mult)
                # y = q0_b * xc  [(j,i),(sg,g,p,d)]
                y = smp.tile([128, 512], f32, tag="y")
                q0b = q0[:].rearrange("p (k o) -> p k o", o=1).broadcast_to(
                    (128, 64, 8))
                y_i = nc.vector.scalar_tensor_tensor(
                    y[:].rearrange("p (k d) -> p k d", d=8), q0b, 0.0,
                    xc4[:].rearrange("p (k d) -> p k d", d=8),
                    OP.add, OP.mult)

                # it0 grad: 16 MMs (j,g), wn stationary
                # GR out [(g,o), sg*128 + j*32 + p*8 + d]
                fdeps = [y_i] + _carry.get("grdr", [])
                if mg == 0:
                    fdeps.append(d_wn4)
                fence = pe_fence(fdeps)
                lgmm = None
                for j in range(4):
                    for g in range(4):
                        for sg in range(4):
                            _gm = nc.tensor.matmul(
                                ps[32 * g:32 * g + 32,
                                   sg * 128 + j * 32:sg * 128 + j * 32 + 32],
                                wn4[32 * j:32 * j + 32, :],
                                y[32 * j:32 * j + 32,
                                  sg * 128 + g * 32:sg * 128 + g * 32 + 32],
                                tile_position=(32 * j, 32 * g),
                            )
                            if fence is not None:
                                add_dep_helper(_gm.ins, fence.ins,
                                               sync=False, reason="order")
                            lgmm = _gm

                aold = x0e
                anew, anew_i, grdr, u2_i = exp_chain(mg, 0, aold, d_x0e, lgmm)
                prev_abd, scat_i = make_abd(anew, anew_i, first=(mg == 0))
                aold = anew

                # ================= iterations 1, 2 =================
                halves = ((0, 1), (2, 3)) if split else ((0, 1, 2, 3),)
                HW_ = 1024 if split else 2048
                for it in (1, 2):
                    # inner matmuls, per g-half
                    fdeps = [scat_i, _carry.get("ccl0"), _carry.get("ccl1")]
                    if it == 1 and mg == 0:
                        fdeps.append(d_x4)
                    fence = pe_fence(fdeps)
                    lims = []
                    for h, gs_ in enumerate(halves):
                        lim = None
                        for g in gs_:
                            for sg in range(n_sg):
                                for j in range(4):
                                    _mm = nc.tensor.matmul(
                                        ps[32 * j:32 * j + 32,
                                           2048 + g * 512 + sg * 128:
                                           2048 + g * 512 + sg * 128 + 128],
                                        x44[32 * g:32 * g + 32,
                                            sg * 128 + j * 32:
                                            sg * 128 + j * 32 + 32],
                                        prev_abd[32 * g:32 * g + 32,
                                                 sg * 512 + j * 128:
                                                 sg * 512 + j * 128 + 128],
                                        tile_position=(32 * g, 32 * j),
                                    )
                                    if fence is not None:
                                        add_dep_helper(_mm.ins, fence.ins,
                                                       sync=False,
                                                       reason="order")
                                    lim = _mm
                        lims.append(lim)

                    # factor chain, pipelined across the two halves
                    ccl_t, c2_t, rs_t, tn_t, at_t, q_t = {}, {}, {}, {}, {}, {}
                    ccl_i, c2_i, rs_i, tn_i, at_i, q_i = {}, {}, {}, {}, {}, {}
                    fw = chp.tile([128, 2048], f32, tag="tD")
                    fw_i = {}
                    for h, gs_ in enumerate(halves):
                        inap = ps[:, 2048 + h * HW_:2048 + (h + 1) * HW_]
                        eng_fence(nc.vector,
                                  [lims[h], _carry.get(f"at{h}")])
                        ccl_t[h] = chp.tile([128, HW_], f32, name=f"ccl{h}", tag=f"tA{h}")
                        ccl_i[h] = nc.vector.tensor_scalar(
                            ccl_t[h][:], inap, float(-CLIP), float(CLIP),
                            OP.max, OP.min)
                        _carry[f"ccl{h}"] = ccl_i[h]
                        c2_t[h] = chp.tile([128, HW_], f32, name=f"c2{h}", tag=f"tB{h}")
                        c2_i[h] = nc.vector.tensor_tensor(
                            c2_t[h][:], ccl_t[h][:], ccl_t[h][:], OP.mult)
                        rs_t[h] = chp.tile([128, HW_], f32, name=f"rs{h}", tag=f"tC{h}")
                        eng_fence(nc.scalar, [c2_i[h]])
                        rs_i[h] = act_raw(rs_t[h][:], c2_t[h][:], AF.Rsqrt,
                                          scale=-1.0, bias=1.0)
                    for h, gs_ in enumerate(halves):
                        tn_t[h] = chp.tile([128, HW_], f32, name=f"tn{h}", tag=f"tB{h}")
                        eng_fence(nc.vector, [rs_i[h]])
                        tn_i[h] = nc.vector.scalar_tensor_tensor(
                            tn_t[h][:], ccl_t[h][:], 1.0, rs_t[h][:],
                            OP.subtract, OP.mult)
                        at_t[h] = chp.tile([128, HW_], f32, name=f"atx{h}", tag=f"tA{h}")
                        eng_fence(nc.scalar, [tn_i[h]])
                        at_i[h] = act_raw(at_t[h][:], tn_t[h][:], AF.Arctan)
                        _carry[f"at{h}"] = at_i[h]
                    gfence = {}
                    for h, gs_ in enumerate(halves):
                        q_t[h] = chp.tile([128, HW_], f32, name=f"q{h}", tag=f"tB{h}")
                        eng_fence(nc.vector, [at_i[h]])
                        q_i[h] = nc.vector.scalar_tensor_tensor(
                            q_t[h][:], at_t[h][:], 2.0, rs_t[h][:],
                            OP.mult, OP.mult)
                        if mg == 0 and it == 1 and h == 0:
                            eng_fence(nc.vector, [d_wnf])
                        fw_i[h] = nc.vector.tensor_tensor(
                            fw[:, h * HW_:(h + 1) * HW_], q_t[h][:],
                            wnf[:, h * HW_:(h + 1) * HW_], OP.mult)
                        # grad matmuls for this half
                        gdeps = [fw_i[h]] + (grdr if h == 0 else [])
                        gfence[h] = pe_fence(gdeps)
                        lgmm = None
                        for g in gs_:
                            for sg in range(n_sg):
                                for j in range(4):
                                    for p in range(4):
                                        _gm = nc.tensor.matmul(
                                            ps[32 * g:32 * g + 32,
                                               sg * 128 + j * 32 + p * 8:
                                               sg * 128 + j * 32 + p * 8 + 8],
                                            fw[32 * j:32 * j + 32,
                                               g * 512 + sg * 128 + p * 32:
                                               g * 512 + sg * 128 + p * 32 + 32],
                                            xc4[32 * j:32 * j + 32,
                                                sg * 128 + g * 32 + p * 8:
                                                sg * 128 + g * 32 + p * 8 + 8],
                                            tile_position=(32 * j, 32 * g),
                                        )
                                        if gfence[h] is not None:
                                            add_dep_helper(_gm.ins,
                                                           gfence[h].ins,
                                                           sync=False,
                                                           reason="order")
                                        lgmm = _gm

                    anew, anew_i, grdr, u2_i = exp_chain(mg, it, aold, None,
                                                         lgmm)
                    if it == 1:
                        prev_abd, scat_i = make_abd(anew, anew_i, first=False)
                    else:
                        eng_fence(nc.sync,
                                  [anew_i, d_xc, d_x4, d_x0c, d_x0e])
                        _carry["outdma"] = nc.sync.dma_start(out_ds[mg][:],
                                                             anew[:])
                        _alldmas.append(_carry["outdma"])
                    aold = anew
                _carry["grdr"] = grdr
                _carry["q"] = u2_i
                _carry["lgmm"] = lgmm
            eng_fence(nc.sync,
                      _alldmas + [lgmm, _carry.get("at1") or _carry.get("at0"),
                                  u2_i, anew_i, _carry.get("lastpool"),
                                  _carry.get("lastact")])

    if strip:
        _strip_redundant_pe_waits(nc)
    return nc


def _strip_redundant_pe_waits(nc):
    """HW wait-slot limits: Matmult=1, engine ops=2 (use 1), DMA=2.  When an
    instruction exceeds its limit, drop (a) waits already covered by an
    earlier same-engine-queue wait (the fence NoOps inserted before each
    phase), then (b) waits on the instruction's own engine semaphore.
    Self-waits are kept when they fit: the DVE pipelines writebacks, so a
    small-op producer read a couple of small ops later needs the explicit
    self-wait the tile scheduler inserted."""
    eng_sem = {"DVE": "DVE_", "Activation": "Activation_", "Pool": "Pool_",
               "PE": "PE_", "SP": "SP_"}
    covered = {}   # (engine, sem id) -> max waited value
    for inst in nc.all_instructions():
        eng = getattr(inst, "engine", None)
        ename = str(eng).split(".")[-1] if eng is not None else ""
        si = inst.sync_info
        if si is None:
            continue
        waits = list(si.on_wait or [])
        if not waits:
            continue
        tname = type(inst).__name__
        if tname in ("InstDrain", "InstEventSemaphore", "InstNoOp"):
            limit = 99
        else:
            limit = 1
        changed = False
        if len(waits) > limit and ename in eng_sem:
            # (a) coverage strip
            nw = [x for x in waits
                  if covered.get((ename, x.id), -(10 ** 9)) < x.wait_value]
            if len(nw) != len(waits):
                waits, changed = nw, True
            # (b) self-engine waits, only if still over limit
            pref = eng_sem.get(ename) if ename in ("DVE", "Activation", "Pool") else None
            if len(waits) > limit and pref and tname != "InstDMACopy":
                nw = [x for x in waits if not x.ant_name.startswith(pref)]
                if len(nw) != len(waits):
                    waits, changed = nw, True
            if len(waits) > limit:
                raise RuntimeError(
                    f"{tname} {inst.name} ({ename}) has {len(waits)} uncovered "
                    f"waits: {[x.ant_name for x in waits]}")
        if changed:
            si.on_wait = waits
        for x in waits:
            if ename in eng_sem and covered.get((ename, x.id), -(10 ** 9)) < x.wait_value:
                covered[(ename, x.id)] = x.wait_value


# --------------------------------------------------------------------------
# entry point
# --------------------------------------------------------------------------
def kernel(x, w):
    from concourse.bass_utils import run_bass_kernel_spmd

    per_core = _host_prep(x, w)
    if "nc" not in _CACHE:
        _CACHE["nc"] = build_bass(split=_CACHE.get("split", True))
    nc = _CACHE["nc"]
    in_maps = [per_core[c] for c in range(NCORES)]
    res = run_bass_kernel_spmd(nc, in_maps, list(range(NCORES)))
    _CACHE["exec_time_ns"] = getattr(res, "exec_time_ns", None)
    outs = [np.stack([res.results[c][f"out{k}"] for k in range(N_MG)])
            for c in range(NCORES)]
    return _host_unpack(outs)


if __name__ == "__main__":
    import sys
    sys.path.insert(0, "/root/problem")
    import reference

    inputs = reference.setup_inputs()
    out = kernel(**{k: np.asarray(v) for k, v in inputs.items()})
    print("kernel output shape:", out.shape, out.dtype)
